# revision 44
# baseline (speedup 1.0000x reference)
"""DSAttention (de-stationary attention) TRN2 Bass kernel.

Computes, per (b, h):
    scores = (q @ k^T) * tau_b + delta_b          [L, S]
    scores = where(causal_mask, -1e9, scores)
    A = softmax(scale * scores)                    (no max-subtraction needed:
                                                    logits are O(10), exp is safe)
    out = A @ v                                    [L, D]

Strategy: batch*head parallel over 8 cores (4 (b,h) pairs per core).
Per (b,h), everything is computed in "transposed score" space:
    X_T[s, l] = sum_e KT[e, s] * QT[e, l]       (QT pre-scaled by scale*tau on host)
    p = exp(X_T + scale*delta[s])               (delta via ACT per-partition bias)
    OutT[d, l] = sum_s V'[s, d] * p[s, l]       (V' has a ones column -> row 64 of
                                                 OutT is the softmax denominator)
then transpose OutT back with the PE and normalize with DVE.

Matmuls run in float32r (TF32-like, full PE rate); fp32 would be 4x slower.
Causality is exploited at 128-column granularity (half the FLOPs skipped);
the diagonal 128x128 blocks are masked by zeroing p with gpsimd affine_select.
"""

import math

import numpy as np

import bass_rust
import concourse.bass as bass
import concourse.mybir as mybir
import concourse.tile as tile
from concourse.bass_utils import run_bass_kernel_spmd

B, L, S, H, E, D = 2, 2048, 2048, 16, 64, 64
NCORES = 8
BH = B * H                      # 32 (b,h) pairs
BH_PER_CORE = BH // NCORES      # 4
SCALE = 1.0 / math.sqrt(E)

F32 = mybir.dt.float32
F32R = mybir.dt.float32r

# packed per-(b,h) input layout, [128, XCOLS]:
#   cols [0, 2048)        rows 0:64 = scale*tau*q^T, row 64 = 1.0, rest 0
#   cols [2048, 4096)     rows 0:64 = k^T, row 64 = scale*delta, rest 0
#   cols [4096, 5136)     V' rearranged: col 4096 + 65*j + c = V'[128j+p, c]
# The QK matmul contracts over all 128 partitions: fp32r runs at half rate
# for K<=64, so the zero-padded K=128 contraction is 2x faster AND row 64
# adds the de-stationary shift (1.0 * scale*delta[s]) into the logits for
# free (no ACT bias needed).
QK_COLS = 2 * L                 # 4096
VP_COLS = (S // 128) * (D + 1)  # 1040
XCOLS = QK_COLS + VP_COLS       # 5136
RCOLS = XCOLS                   # everything is rounded to f32r


class _SplitDrainTileContext(tile.TileContext):
    """This walrus build rejects instructions carrying more than one sem
    wait; the kernel-tail drain aggregates one wait per active processor.
    Split them across a chain of drains on the same engine."""

    def _drain_and_barrier(self, tick_clock, wait_clock):
        nc = self.nc
        drain_inst = nc.sync.drain()
        wait_clock.add_sem_waits(
            drain_inst.ins, bass_rust.ScopedClock({None: tick_clock.global_clock})
        )
        si = drain_inst.ins.sync_info
        waits = list(si.on_wait) if si is not None and si.on_wait else []
        if len(waits) > 1:
            si.on_wait = waits[:1]
            for w in waits[1:]:
                d2 = nc.sync.drain()
                d2.ins.sync_info = bass_rust.SyncInfo(on_wait=[w], on_update=[])
        nc.all_engine_barrier()
        popped = nc._tile_sem_poison_stack.pop()
        assert popped is self._sem_poison
        nc.clear_and_free_semaphores(list(self.sems.allocated().values()))
        nc.all_engine_barrier()


def _legalize_waits(nc, max_waits=1):
    """This walrus build rejects instructions with more than `max_waits`
    sem waits. Spill extras onto same-engine NoOps inserted just before
    the offending instruction (same-engine program order preserves the
    wait semantics)."""
    for f in nc.m.functions:
        for bb in f.blocks:
            insts = bb.instructions
            for idx in range(len(insts) - 1, -1, -1):
                inst = insts[idx]
                si = getattr(inst, "sync_info", None)
                if si is None or not si.on_wait:
                    continue
                ow = list(si.on_wait)
                sem = [w for w in ow if w.sync_type == "semaphore"]
                other = [w for w in ow if w.sync_type != "semaphore"]
                budget = max(0, max_waits - len(other))
                if len(sem) <= budget:
                    continue
                keep, spill = sem[:budget], sem[budget:]
                si.on_wait = other + keep
                for w in reversed(spill):
                    n = mybir.InstNoOp(name=f"W-{nc.next_id()}", ins=[], outs=[])
                    n.engine = inst.engine
                    n.sync_info = bass_rust.SyncInfo(on_wait=[w], on_update=[])
                    nc.register_instruction(n, overwrite=True)
                    insts.insert(idx, n)


def _chunks(lo, hi, bank=512):
    """Split [lo, hi) at multiples of `bank` (PSUM bank boundaries)."""
    out = []
    c = lo
    while c < hi:
        c1 = min((c // bank + 1) * bank, hi)
        out.append((c, c1))
        c = c1
    return out


def _build_program():
    nc = bass.Bass("TRN2", target_bir_lowering=False, debug=False)
    # Inputs are pre-rounded to f32r on the host (13-bit RNE mantissa zeroing,
    # verified against the HW tensor_copy rounding) and DMA'd straight into
    # the f32r compute tile — no on-chip cast needed.
    # qk part: only rows 0:65 shipped (row 64 = aug row); rows 65:128 are
    # zeroed on-chip once per xr slot. vp part ships all 128 rows.
    x_d = nc.declare_dram_parameter("x", [BH_PER_CORE, 65, QK_COLS], F32R, isOutput=False)
    v_d = nc.declare_dram_parameter("v", [BH_PER_CORE, 128, VP_COLS], F32R, isOutput=False)
    # output stays in the transposed orientation: [bh, quarter, d, l_rel];
    # row d == D is the softmax denominator; the host divides + transposes.
    o_d = nc.declare_dram_parameter("o", [BH_PER_CORE, 4, D + 1, 512], F32, isOutput=True)

    with _SplitDrainTileContext(nc) as tc:
        with (
            tc.tile_pool(name="xin", bufs=2) as in_pool,
            tc.tile_pool(name="xr", bufs=1) as r_pool,
            tc.tile_pool(name="p", bufs=3) as p_pool,
            tc.tile_pool(name="otsb", bufs=2) as otsb_pool,
            tc.tile_pool(name="strip_ps", bufs=2, space="PSUM") as strip_ps_pool,
            tc.tile_pool(name="out_ps", bufs=2, space="PSUM") as out_ps_pool,
        ):
            # prefetch + round all per-bh inputs up front (keeps the PE fed
            # across bh boundaries; xr slots are per-bh, never reused)
            xrs = []
            for i in range(BH_PER_CORE):
                xr = r_pool.tile([128, RCOLS], F32R, name=f"xr{i}", tag=f"xr{i}")
                # zero the contraction pad rows (uint32 view dodges the f32r
                # ISA check); the 65-row DMA below overwrites row 64 after.
                # Split across DVE + gpsimd so the two halves run in parallel.
                nc.vector.memset(xr[64:128, 0:L].bitcast(mybir.dt.uint32), 0)
                nc.gpsimd.memset(xr[64:128, L:QK_COLS].bitcast(mybir.dt.uint32), 0)
                xrs.append(xr)
            for i in range(BH_PER_CORE):
                xr = xrs[i]
                nc.sync.dma_start(out=xr[0:65, 0:QK_COLS], in_=x_d[i])
                nc.sync.dma_start(out=xr[:, QK_COLS:XCOLS], in_=v_d[i])

            for i in range(BH_PER_CORE):
                xr = xrs[i]
                qt = xr[:, 0:L]                # [128, 2048] scaled q^T (padded)
                kt = xr[:, L:2 * L]            # [128, 2048] k^T (padded)
                vp = xr[:, QK_COLS:XCOLS]      # [128, 1040]

                for qr in range(4):
                    l0 = 512 * qr
                    jmax = 4 * qr + 3
                    js = list(range(jmax + 1))
                    ot_ps = out_ps_pool.tile([D + 1, 512], F32)
                    # group up to 3 j-strips per PSUM tile -> one ACTIVATE
                    # covers all of them (the 352-cycle ACT overhead is the
                    # single biggest fixed cost in the kernel)
                    for g0 in range(0, len(js), 3):
                        grp = js[g0:g0 + 3]
                        xt_ps = strip_ps_pool.tile([128, 1536], F32)
                        lens = []
                        for s, j in enumerate(grp):
                            s0 = 128 * j
                            ls = max(l0, s0)
                            ln = l0 + 512 - ls
                            lens.append(ln)
                            # extend non-final short pieces to the full bank so
                            # the group ACTIVATE reads no uninitialized PSUM
                            # (the extra columns are junk and never consumed;
                            # rhs comes from the full xr view since ls+512 may
                            # run past the qt region into kt — also junk)
                            n_mm = 512 if s < len(grp) - 1 else ln
                            nc.tensor.matmul(
                                xt_ps[:, 512 * s:512 * s + n_mm],
                                lhsT=kt[:, s0:s0 + 128],
                                rhs=xr[:, ls:ls + n_mm],
                                start=True, stop=True,
                            )
                        p = p_pool.tile([128, 1536], F32R)
                        width = 512 * (len(grp) - 1) + lens[-1]
                        nc.scalar.activation(
                            p[:, 0:width], xt_ps[:, 0:width],
                            mybir.ActivationFunctionType.Exp,
                        )
                        for s, j in enumerate(grp):
                            s0 = 128 * j
                            if s0 >= l0:
                                # diagonal block: zero p where s > l
                                # (keep where (l - s) >= 0)
                                nc.gpsimd.affine_select(
                                    out=p[:, 512 * s:512 * s + 128],
                                    in_=p[:, 512 * s:512 * s + 128],
                                    compare_op=mybir.AluOpType.is_ge, fill=0.0,
                                    base=0, channel_multiplier=-1,
                                    pattern=[[1, 128]],
                                )
                            rel = max(0, 128 * j - l0)
                            nc.tensor.matmul(
                                ot_ps[:, rel:512],
                                lhsT=vp[:, (D + 1) * j:(D + 1) * (j + 1)],
                                rhs=p[:, 512 * s:512 * s + (512 - rel)],
                                start=(j == 0), stop=(j == jmax),
                            )
                    # epilogue: evacuate PSUM and ship raw (numerator rows +
                    # denominator row); the host divides and un-transposes.
                    ot_sb = otsb_pool.tile([D + 1, 512], F32)
                    nc.vector.tensor_copy(ot_sb, ot_ps)
                    nc.sync.dma_start(out=o_d[i, qr], in_=ot_sb)
    _legalize_waits(nc)
    return nc


_PROGRAM = None


def _get_program():
    global _PROGRAM
    if _PROGRAM is None:
        _PROGRAM = _build_program()
    return _PROGRAM


def _round_f32r(a):
    """Round fp32 to the f32r grid (13 low mantissa bits zeroed, RNE) —
    matches the PE's fp32r operand format."""
    b = a.astype(np.float32).view(np.uint32)
    r = (b + np.uint32(0x0FFF) + ((b >> np.uint32(13)) & np.uint32(1))) & ~np.uint32(0x1FFF)
    return r.view(np.float32)


def _prepare_inputs(q, k, v, tau, delta):
    """Pack full inputs into the per-core [4, 128, XCOLS] device layout."""
    qs = (q.astype(np.float64) * (SCALE * tau.astype(np.float64))[:, 0, None, None, None]).astype(np.float32)
    # [B,L,H,E] -> [BH, E, L]
    qt = np.ascontiguousarray(qs.transpose(0, 2, 3, 1).reshape(BH, E, L))
    kt = np.ascontiguousarray(k.transpose(0, 2, 3, 1).reshape(BH, E, S))
    # V' = [v, 1]: [BH, S, D+1] -> [BH, 128, 16*(D+1)]
    vt = v.transpose(0, 2, 1, 3).reshape(BH, S, D)
    vp = np.concatenate([vt, np.ones((BH, S, 1), np.float32)], axis=2)
    vp = np.ascontiguousarray(
        vp.reshape(BH, S // 128, 128, D + 1).transpose(0, 2, 1, 3).reshape(BH, 128, VP_COLS)
    )
    dsc = (SCALE * delta).astype(np.float32)  # [B, S]

    x = np.empty((BH, E + 1, QK_COLS), np.float32)
    x[:, 0:E, 0:L] = qt
    x[:, E, 0:L] = 1.0
    x[:, 0:E, L:2 * L] = kt
    x[:, E, L:2 * L] = np.repeat(dsc, H, axis=0)
    return _round_f32r(x), _round_f32r(vp)


def _numpy_fallback(q, k, v, att_mask, tau, delta):
    out = np.empty((B, L, H, D), np.float32)
    mask = att_mask[:, 0]  # [B, L, S]
    for b in range(B):
        for h in range(H):
            s = (q[b, :, h, :] @ k[b, :, h, :].T) * tau[b, 0] + delta[b][None, :]
            s = np.where(mask[b], -1e9, s).astype(np.float32)
            s = SCALE * s
            s = s - s.max(axis=-1, keepdims=True)
            e = np.exp(s)
            a = e / e.sum(axis=-1, keepdims=True)
            out[b, :, h, :] = a @ v[b, :, h, :]
    return out


def kernel(q, k, v, att_mask, tau, delta):
    q = np.asarray(q, np.float32)
    k = np.asarray(k, np.float32)
    v = np.asarray(v, np.float32)
    tau = np.asarray(tau, np.float32)
    delta = np.asarray(delta, np.float32)
    att_mask = np.asarray(att_mask)

    causal = np.triu(np.ones((L, S), bool), k=1)
    if not all(np.array_equal(att_mask[b, 0], causal) for b in range(B)):
        return _numpy_fallback(q, k, v, att_mask, tau, delta)

    x, vp = _prepare_inputs(q, k, v, tau, delta)
    nc = _get_program()
    in_maps = [
        {
            "x": np.ascontiguousarray(x[c * BH_PER_CORE:(c + 1) * BH_PER_CORE]),
            "v": np.ascontiguousarray(vp[c * BH_PER_CORE:(c + 1) * BH_PER_CORE]),
        }
        for c in range(NCORES)
    ]
    res = run_bass_kernel_spmd(nc, in_maps, list(range(NCORES))).results

    out = np.empty((B, L, H, D), np.float32)
    for c in range(NCORES):
        o = res[c]["o"]  # [4, 4, D+1, 512]: raw numerators + denominator row
        norm = o[:, :, 0:D, :] / o[:, :, D:D + 1, :]
        for i in range(BH_PER_CORE):
            bh = c * BH_PER_CORE + i
            out[bh // H, :, bh % H, :] = norm[i].transpose(0, 2, 1).reshape(L, D)
    return out
